# revision 38
# baseline (speedup 1.0000x reference)
"""CompositionalAttention Trainium2 kernel (8 NeuronCores, SPMD).

Shapes (hardcoded): query (T=2048, B=2, E=1024), H=16 heads, R=8 rules,
HD=64, VD=128. Output (T, B, E) float32.

Sharding: (batch x t-quarter) -> 8 cores. Core c handles b = c//4 and the
t-slice [tq*512, (tq+1)*512) with tq = c%4, computing ALL heads for that
slice so the output projection needs no cross-core reduction. Each core
returns its exact (512, 1024) slice of the final output.

Algebraic simplification used (verified vs reference to 2.5e-6):
the rule-selection softmax input is
    score[b,h,t,r] = v_q . w_sel + bsc + attn[b,h,t,r,:] . w_vd
and the first two terms are constant in r, so they cancel in the softmax
over r. Wvq/bvq/Wsc[:, :SEL]/bsc never affect the output. Further, with
unnormalized attention A~_r = P~ @ v_r (P~ = exp(logits), d = P~ @ 1):
    g_r = (P~ @ u_r) / d        with u_r = v_r @ w_vd  (folded into V proj)
    sel = softmax_r(g);  out_h = sum_r (sel_r / d) * A~_r

Perf notes (v2 vs v1 baseline):
 - consolidated SBUF input tiles + one multi-dim DMA per tensor piece,
   spread over 4 engine queues (sync/scalar/gpsimd/vector) so the
   per-descriptor issue cost (~600ns) doesn't serialize the input stream
 - PE warm-up matmuls at t=0 so HAM reaches K=8/8 before real work and
   the projection matmuls never run at the cold 1.2 GHz clock
 - logits matmuls for head h+1 are emitted interleaved into the last
   t-chunk of head h's combine (and head 0's into the k-projection), so
   the per-head logits burst never stalls the PE on the exp activations
 - qp[h] (zero-padded per-head q) and oa[h] (transposed combine output)
   share one SBUF tile: qp dies exactly when oa's writes begin
"""

import numpy as np
from contextlib import ExitStack

import ml_dtypes
import concourse.bass as bass
import concourse.bacc as bacc
import concourse.mybir as mybir
from concourse import tile
from concourse.bass_utils import run_bass_kernel_spmd

AF = mybir.ActivationFunctionType
ALU = mybir.AluOpType
F32 = mybir.dt.float32

T, B, E, H, R = 2048, 2, 1024, 16, 8
HD, VD, SEL = 64, 128, 64
TS = T // 4            # 512 t-rows per core
NK = E // 128          # 8 contraction chunks over E
NS = T // 128          # 16 s-chunks
NT = TS // 128         # 4 t-chunks per core
VW = R * VD            # 1024 v columns
VX = VW + R            # 1032: v columns + 8 u columns
NCORES = 8
NWARM = 56             # PE warm-up matmuls (N=128) to get HAM to K=8/8

DT = mybir.dt.bfloat16
NPDT = ml_dtypes.bfloat16


def _build():
    nc = bacc.Bacc("TRN2", target_bir_lowering=False, debug=False,
                   num_devices=NCORES)
    qt = nc.declare_dram_parameter("qt_full", [E, T], DT, isOutput=False)
    qts_d = nc.declare_dram_parameter("qt_slice", [E, TS], DT, isOutput=False)
    wqt = nc.declare_dram_parameter("wqt", [E, E], DT, isOutput=False)
    bq_col = nc.declare_dram_parameter("bq_col", [E, 1], F32, isOutput=False)
    wkt = nc.declare_dram_parameter("wkt", [E, E], DT, isOutput=False)
    bk_col = nc.declare_dram_parameter("bk_col", [E, 1], F32, isOutput=False)
    wvt = nc.declare_dram_parameter("wvt", [E, VX], DT, isOutput=False)
    vbias = nc.declare_dram_parameter("vbias", [128, VX], F32, isOutput=False)
    wot = nc.declare_dram_parameter("wot", [H * VD, E], DT, isOutput=False)
    bo_bc = nc.declare_dram_parameter("bo_bc", [128, E], F32, isOutput=False)
    ident = nc.declare_dram_parameter("ident", [128, 128], DT, isOutput=False)
    out = nc.declare_dram_parameter("out", [TS, E], F32, isOutput=True)

    # DRAM views reshaped so one DMA covers all row-chunks: [p, chunk, col]
    qt_v = qt[:, :].rearrange("(k p) t -> p k t", k=NK)
    qts_v = qts_d[:, :].rearrange("(k p) t -> p k t", k=NK)
    wq_v = wqt[:, :].rearrange("(k p) e -> p k e", k=NK)
    wk_v = wkt[:, :].rearrange("(k p) e -> p k e", k=NK)
    wv_v = wvt[:, :].rearrange("(k p) e -> p k e", k=NK)
    wo_v = wot[:, :].rearrange("(k p) e -> p k e", k=H)
    bq_v = bq_col[:, :].rearrange("(k p) o -> p (k o)", k=NK)
    bk_v = bk_col[:, :].rearrange("(k p) o -> p (k o)", k=NK)

    with ExitStack() as ctx:
        tc = ctx.enter_context(tile.TileContext(nc))
        pers = ctx.enter_context(tc.tile_pool(name="pers", bufs=1))

        # ---- persistent SBUF tensors ----
        kt = [pers.tile([128, T], DT, tag=f"kt{m}", name=f"kt{m}") for m in range(NK)]
        # per-head zero-padded q (K=128 so FWL engages); after head h's
        # logits are done the same tile is re-used for oa[h] (the
        # transposed combine output feeding the out-projection lhsT)
        qoa = [pers.tile([128, TS], DT, tag=f"qoa{h}", name=f"qoa{h}") for h in range(H)]
        va = [pers.tile([128, VX + 1], DT, tag=f"va{s}", name=f"va{s}") for s in range(NS)]
        bq_sb = pers.tile([128, NK], F32, tag="bq", name="bq")
        bk_sb = pers.tile([128, NK], F32, tag="bk", name="bk")
        vb_sb = pers.tile([128, VX], F32, tag="vb", name="vb")
        bo_sb = pers.tile([128, E], F32, tag="bo", name="bo")
        id_sb = pers.tile([128, 128], DT, tag="id", name="id")
        wup = pers.tile([128, 128], DT, tag="wup", name="wup")
        dmagate = pers.tile([1, 8], DT, tag="dmagate", name="dmagate")

        # et (exp of logits) tiles cycle through this pool; 26 bufs >= 24
        # simultaneously-live tiles (16 of head h + 8 of head h+1 emitted
        # during head h's t=2 chunk, before any of head h's tiles free)
        exP = ctx.enter_context(tc.tile_pool(name="exP", bufs=26))
        # logits psum pool lives from the k-projection (head 0 interleave)
        # through all of phase B; SBUF and PSUM allocators are independent
        # so this outlives the phase-A SBUF staging pool below
        plt = ctx.enter_context(tc.tile_pool(name="plt", bufs=2,
                                             space="PSUM"))
        phA_ctx = tc.tile_pool(name="phA", bufs=1)
        phA = phA_ctx.__enter__()
        qts = phA.tile([128, NK, TS], DT, tag="qts", name="qts")
        qtall = phA.tile([128, NK, T], DT, tag="qtall", name="qtall")
        wqall = phA.tile([128, NK, E], DT, tag="wqall", name="wqall")
        wkall = phA.tile([128, NK, E], DT, tag="wkall", name="wkall")
        wvall = phA.tile([128, NK, VX], DT, tag="wvall", name="wvall")

        nc.vector.memset(wup[:], 0.0)

        # ---- input DMAs ----
        # The three hwdge queues share ~330 GB/s aggregate, and each queue
        # transfers strictly in issue order -- so order IS priority. The
        # critical chain is qts -> wq (q-proj) -> wk + qt quarters
        # (k-proj, nb-outer); wv/wo/biases are needed much later.
        nc.sync.dma_start(qts[:], qts_v)
        nc.scalar.dma_start(wqall[:, :, 0:128], wq_v[:, :, 0:128])
        nc.gpsimd.dma_start(bq_sb[:], bq_v)
        nc.gpsimd.dma_start(bk_sb[:], bk_v)
        nc.gpsimd.dma_start(id_sb[:], ident[:, :])
        nc.scalar.dma_start(wqall[:, :, 128:512], wq_v[:, :, 128:512])
        nc.scalar.dma_start(wqall[:, :, 512:768], wq_v[:, :, 512:768])
        nc.scalar.dma_start(wqall[:, :, 768:E], wq_v[:, :, 768:E])
        nc.gpsimd.dma_start(qtall[:, :, 0:TS], qt_v[:, :, 0:TS])
        nc.sync.dma_start(wkall[:, :, 0:512], wk_v[:, :, 0:512])
        nc.gpsimd.dma_start(qtall[:, :, TS:2 * TS], qt_v[:, :, TS:2 * TS])
        nc.sync.dma_start(wkall[:, :, 512:E], wk_v[:, :, 512:E])
        nc.sync.dma_start(bo_sb[:], bo_bc[:, :])

        # ---- PE warm-up: get HAM to K=8/8 while input DMAs stream ----
        with tc.tile_pool(name="pwu", bufs=2, space="PSUM") as pwu:
            for i in range(NWARM):
                pw = pwu.tile([128, 128], F32, tag="pw", name="pw")
                nc.tensor.matmul(pw[:], lhsT=wup[:], rhs=wup[:],
                                 start=True, stop=True)

        # ---- logits emitter machinery ----
        plt_pool = [None]
        et_store = [[None] * NS for _ in range(H)]

        def logits_ops(h):
            m2 = h // 2
            ops = []
            for s in range(NS):
                def op(h=h, s=s, m2=m2):
                    psl = plt_pool[0].tile([128, TS], F32, tag="plt",
                                           name="psl")
                    nc.tensor.matmul(
                        psl[:],
                        lhsT=kt[m2][:, s * 128:(s + 1) * 128],
                        rhs=qoa[h][:],
                        start=True, stop=True)
                    et = exP.tile([128, TS], DT, tag="et", name="et")
                    nc.scalar.activation(et[:], psl[:], AF.Exp)
                    et_store[h][s] = et
                ops.append(op)
            return ops

        # ---- Phase A: projections ----
        plt_pool[0] = plt
        with tc.tile_pool(name="ppk", bufs=2, space="PSUM") as ppk:
            # q projection -> per-head zero-padded tiles
            for m in range(NK):
                c0, c1 = m * 128, (m + 1) * 128
                ps = ppk.tile([128, 512], F32, tag="ppk", name="ppk")
                for k in range(NK):
                    nc.tensor.matmul(
                        ps[:], lhsT=wqall[:, k, c0:c1],
                        rhs=qts[:, k, :],
                        start=(k == 0), stop=(k == NK - 1))
                h0, h1 = 2 * m, 2 * m + 1
                nc.vector.memset(qoa[h0][64:128, :], 0.0)
                nc.vector.memset(qoa[h1][0:64, :], 0.0)
                nc.scalar.activation(qoa[h0][0:64, :], ps[0:64, :],
                                     AF.Identity,
                                     bias=bq_sb[0:64, m:m + 1])
                nc.scalar.activation(qoa[h1][64:128, :], ps[64:128, :],
                                     AF.Identity,
                                     bias=bq_sb[64:128, m:m + 1])

            # the remaining bulk input (qt q2/q3, wv, vb: ~4.6MB, not
            # needed until ~60us+) is held back behind a marker op that
            # waits for a q-proj output: the 3 queues share ~330 GB/s,
            # so letting this bulk run at t=0 starves the critical
            # qts/wq stream (measured 14us PE stall)
            nc.gpsimd.tensor_copy(dmagate[:], qoa[5][0:1, 0:8])
            nc.gpsimd.dma_start(qtall[:, :, 2 * TS:3 * TS],
                                qt_v[:, :, 2 * TS:3 * TS])
            nc.gpsimd.dma_start(qtall[:, :, 3 * TS:T], qt_v[:, :, 3 * TS:T])
            nc.gpsimd.dma_start(wvall[:, :, 0:516], wv_v[:, :, 0:516])
            nc.gpsimd.dma_start(wvall[:, :, 516:VX], wv_v[:, :, 516:VX])
            nc.gpsimd.dma_start(vb_sb[:], vbias[:, :])

            # kT_all (E_out on partitions, s free); nb-outer so the pass
            # over all m-chunks starts after just ONE qt quarter arrives.
            # head-0 logits op s (s-chunk s, needing kt[0]'s nb=s//4
            # block) interleaves at (nb=s//4, m odd).
            h0ops = logits_ops(0)
            for nb in range(T // 512):
                for m in range(NK):
                    c0, c1 = m * 128, (m + 1) * 128
                    ps = ppk.tile([128, 512], F32, tag="ppk", name="ppk")
                    for k in range(NK):
                        nc.tensor.matmul(
                            ps[:], lhsT=wkall[:, k, c0:c1],
                            rhs=qtall[:, k, nb * 512:(nb + 1) * 512],
                            start=(k == 0), stop=(k == NK - 1))
                    nc.scalar.activation(
                        kt[m][:, nb * 512:(nb + 1) * 512],
                        ps[:], AF.Identity, bias=bk_sb[:, m:m + 1])
                    if m % 2 == 1 and h0ops:
                        h0ops.pop(0)()

        # v_all (s on partitions): [v | u] + bias, plus ones col
        with tc.tile_pool(name="ppv", bufs=2, space="PSUM") as ppv:
            for s in range(NS):
                c0, c1 = s * 128, (s + 1) * 128
                psv = ppv.tile([128, VX], F32, tag="ppv", name="ppv")
                for k in range(NK):
                    lhs = qtall[:, k, c0:c1]
                    nc.tensor.matmul(psv[:, 0:512], lhsT=lhs,
                                     rhs=wvall[:, k, 0:512],
                                     start=(k == 0), stop=(k == NK - 1))
                    nc.tensor.matmul(psv[:, 512:1024], lhsT=lhs,
                                     rhs=wvall[:, k, 512:1024],
                                     start=(k == 0), stop=(k == NK - 1))
                    nc.tensor.matmul(psv[:, 1024:VX], lhsT=lhs,
                                     rhs=wvall[:, k, 1024:VX],
                                     start=(k == 0), stop=(k == NK - 1))
                nc.vector.tensor_add(va[s][:, 0:VX], psv[:], vb_sb[:])
                nc.vector.memset(va[s][:, VX:VX + 1], 1.0)

        # phase-A SBUF staging (qt/wq/wk/wv, ~88KB/partition) dies here,
        # making room for wo + phase-B pools
        phA_ctx.__exit__(None, None, None)

        # ---- load Wo^T (needed in phase C; overlaps phase B) ----
        woP = ctx.enter_context(tc.tile_pool(name="woP", bufs=1))
        woall = woP.tile([128, H, E], DT, tag="wo", name="wo")
        nc.scalar.dma_start(woall[:, 0:8, :], wo_v[:, 0:8, :])
        nc.scalar.dma_start(woall[:, 8:H, :], wo_v[:, 8:H, :])

        # ---- Phase B: attention per head ----
        with (
            tc.tile_pool(name="pa", bufs=2, space="PSUM") as pa,
            tc.tile_pool(name="pas", bufs=1, space="PSUM") as pas,
            tc.tile_pool(name="pt", bufs=1, space="PSUM") as pt,
            tc.tile_pool(name="sm", bufs=4) as sm,
            tc.tile_pool(name="ocp", bufs=4) as ocp,
            tc.tile_pool(name="ob", bufs=2) as obp,
        ):
            def out_proj_group(t):
                # phase C folded into phase B: psums come from the (by
                # now idle) logits pool, and the t=0 group is emitted
                # before the final transpose flush so the PE never waits
                # on the last combine's DVE chain
                t0, t1 = t * 128, (t + 1) * 128
                for e in range(E // 512):
                    pso = plt.tile([128, 512], F32, tag="plt", name="pso")
                    for k in range(H):
                        nc.tensor.matmul(
                            pso[:], lhsT=qoa[k][:, t0:t1],
                            rhs=woall[:, k, e * 512:(e + 1) * 512],
                            start=(k == 0), stop=(k == H - 1))
                    ob = obp.tile([128, 512], F32, tag="ob", name="ob")
                    if t == NT - 1:
                        # last chunk: halve the bias-add/store so the
                        # final DMAs overlap the add and spread queues
                        engs = (nc.sync, nc.gpsimd) if e == 0 \
                            else (nc.scalar, nc.sync)
                        for half, heng in enumerate(engs):
                            c0h = e * 512 + half * 256
                            nc.vector.tensor_add(
                                ob[:, half * 256:(half + 1) * 256],
                                pso[:, half * 256:(half + 1) * 256],
                                bo_sb[:, c0h:c0h + 256])
                            heng.dma_start(
                                out[t0:t1, c0h:c0h + 256],
                                ob[:, half * 256:(half + 1) * 256])
                    else:
                        nc.vector.tensor_add(ob[:], pso[:],
                                             bo_sb[:, e * 512:(e + 1) * 512])
                        eng = nc.sync if e == 0 else nc.scalar
                        eng.dma_start(out[t0:t1, e * 512:(e + 1) * 512],
                                      ob[:])

            def issue_transpose(hh, tt, octile):
                ptr = pt.tile([128, 128], DT, tag="ptr", name="ptr")
                nc.tensor.transpose(ptr[:], octile[:], id_sb[:])
                # psum->sbuf copy on vector (gpsimd can't read PSUM):
                # keeps the scalar engine free for the exp activations
                # that gate the logits psum pool
                nc.vector.tensor_copy(
                    qoa[hh][:, tt * 128:(tt + 1) * 128], ptr[:])

            pending = []
            for h in range(H):
                nxt = logits_ops(h + 1) if h + 1 < H else []
                for t in range(NT):
                    t0, t1 = t * 128, (t + 1) * 128
                    psa = pa.tile([128, VW], F32, tag="psa", name="psa")
                    pss = pas.tile([128, R + 1], F32, tag="pss",
                                   name="pss")
                    for s in range(NS):
                        lhs = et_store[h][s][:, t0:t1]
                        st, sp = (s == 0), (s == NS - 1)
                        nc.tensor.matmul(psa[:, 0:512], lhsT=lhs,
                                         rhs=va[s][:, 0:512],
                                         start=st, stop=sp)
                        nc.tensor.matmul(psa[:, 512:1024], lhsT=lhs,
                                         rhs=va[s][:, 512:1024],
                                         start=st, stop=sp)
                        nc.tensor.matmul(pss[:], lhsT=lhs,
                                         rhs=va[s][:, 1024:VX + 1],
                                         start=st, stop=sp)
                        # next head's logits: 1 per 2 s-steps over the
                        # last two t-chunks (32 slots for 16 ops) so
                        # the ~670ns exp activations keep pace with
                        # the 2-bank logits psum pool
                        if t >= NT - 2 and s % 2 == 1 and nxt:
                            nxt.pop(0)()
                    # selection weights: w_r = softmax_r(G~_r/d) / d
                    rcp_d = sm.tile([128, 1], F32, tag="rcpd",
                                    name="rcpd")
                    nc.vector.reciprocal(rcp_d[:], pss[:, R:R + 1])
                    g = sm.tile([128, R], F32, tag="g", name="g")
                    nc.vector.tensor_scalar_mul(g[:], pss[:, 0:R],
                                                rcp_d[:])
                    selw = sm.tile([128, R], F32, tag="selw", name="selw")
                    ssum = sm.tile([128, 1], F32, tag="ssum", name="ssum")
                    nc.scalar.activation(selw[:], g[:], AF.Exp,
                                         accum_out=ssum[:])
                    den = sm.tile([128, 1], F32, tag="den", name="den")
                    nc.vector.tensor_scalar_mul(den[:], ssum[:],
                                                pss[:, R:R + 1])
                    rcp2 = sm.tile([128, 1], F32, tag="rcp2", name="rcp2")
                    nc.vector.reciprocal(rcp2[:], den[:])
                    w = sm.tile([128, R], F32, tag="w", name="w")
                    nc.vector.tensor_scalar_mul(w[:], selw[:], rcp2[:])
                    # combine rules: out_tile = sum_r w_r * A~_r
                    acc = sm.tile([128, 128], F32, tag="acc", name="acc")
                    nc.vector.tensor_scalar_mul(acc[:], psa[:, 0:128],
                                                w[:, 0:1])
                    for r in range(1, R - 1):
                        acc2 = sm.tile([128, 128], F32, tag="acc",
                                       name="acc")
                        nc.vector.scalar_tensor_tensor(
                            acc2[:], psa[:, r * 128:(r + 1) * 128],
                            w[:, r:r + 1], acc[:],
                            op0=ALU.mult, op1=ALU.add)
                        acc = acc2
                    octile = ocp.tile([128, VD], DT, tag="oc", name="oc")
                    nc.vector.scalar_tensor_tensor(
                        octile[:], psa[:, (R - 1) * 128:R * 128],
                        w[:, R - 1:R], acc[:], op0=ALU.mult, op1=ALU.add)
                    if pending:
                        issue_transpose(*pending.pop(0))
                    pending.append((h, t, octile))
            # t=0 out-projection first: it only needs the (already
            # copied) t=0 slices, and runs while the last combine's DVE
            # chain + final transpose drain
            out_proj_group(0)
            for hh, tt, octile in pending:
                issue_transpose(hh, tt, octile)
            for t in range(1, NT):
                out_proj_group(t)
    nc.finalize()
    return nc


_NC_CACHE = None


def _get_nc():
    global _NC_CACHE
    if _NC_CACHE is None:
        _NC_CACHE = _build()
    return _NC_CACHE


def _prep_in_maps(query, Wq, bq, Wk, bk, Wv, bv, Wsc, Wo, bo):
    scale = np.float32(HD ** -0.5)
    w_vd = Wsc[0, SEL:].astype(np.float32)          # (VD,)

    wqt = np.ascontiguousarray((Wq * scale).T).astype(NPDT)
    bq_col = (bq * scale).reshape(E, 1).astype(np.float32)
    wkt = np.ascontiguousarray(Wk.T).astype(NPDT)
    bk_col = bk.reshape(E, 1).astype(np.float32)

    WvT = np.ascontiguousarray(Wv.T).astype(np.float32)      # (E, VW)
    U_w = np.einsum("erd,d->er", WvT.reshape(E, R, VD), w_vd)  # (E, R)
    wvt = np.concatenate([WvT, U_w], axis=1).astype(NPDT)    # (E, VX)
    ubias = np.einsum("rd,d->r", bv.reshape(R, VD), w_vd)    # (R,)
    vb_row = np.concatenate([bv.astype(np.float32), ubias.astype(np.float32)])
    vbias = np.ascontiguousarray(
        np.broadcast_to(vb_row, (128, VX))).astype(np.float32)

    wot = np.ascontiguousarray(Wo.T).astype(NPDT)            # (H*VD, E)
    bo_bc = np.ascontiguousarray(
        np.broadcast_to(bo, (128, E))).astype(np.float32)
    ident = np.eye(128, dtype=NPDT)

    shared = dict(wqt=wqt, bq_col=bq_col, wkt=wkt, bk_col=bk_col, wvt=wvt,
                  vbias=vbias, wot=wot, bo_bc=bo_bc, ident=ident)

    in_maps = []
    for c in range(NCORES):
        b, tq = c // 4, c % 4
        qT = np.ascontiguousarray(query[:, b, :].T).astype(NPDT)  # (E, T)
        m = dict(shared)
        m["qt_full"] = qT
        m["qt_slice"] = np.ascontiguousarray(qT[:, tq * TS:(tq + 1) * TS])
        in_maps.append(m)
    return in_maps


def kernel(query, Wq, bq, Wk, bk, Wv, bv, Wvq, bvq, Wsc, bsc, Wo, bo,
           _trace=False, _tmpdir=None):
    query = np.asarray(query, np.float32)
    in_maps = _prep_in_maps(
        np.asarray(query, np.float32), np.asarray(Wq, np.float32),
        np.asarray(bq, np.float32), np.asarray(Wk, np.float32),
        np.asarray(bk, np.float32), np.asarray(Wv, np.float32),
        np.asarray(bv, np.float32), np.asarray(Wsc, np.float32),
        np.asarray(Wo, np.float32), np.asarray(bo, np.float32))
    nc = _get_nc()
    res = run_bass_kernel_spmd(nc, in_maps, list(range(NCORES)),
                               trace=_trace, tmpdir=_tmpdir)
    out = np.empty((T, B, E), np.float32)
    for c in range(NCORES):
        b, tq = c // 4, c % 4
        out[tq * TS:(tq + 1) * TS, b, :] = res.results[c]["out"]
    kernel._last_results = res
    return out


# revision 41
# speedup vs baseline: 1.0012x; 1.0012x over previous
"""CompositionalAttention Trainium2 kernel (8 NeuronCores, SPMD).

Shapes (hardcoded): query (T=2048, B=2, E=1024), H=16 heads, R=8 rules,
HD=64, VD=128. Output (T, B, E) float32.

Sharding: (batch x t-quarter) -> 8 cores. Core c handles b = c//4 and the
t-slice [tq*512, (tq+1)*512) with tq = c%4, computing ALL heads for that
slice so the output projection needs no cross-core reduction. Each core
returns its exact (512, 1024) slice of the final output.

Algebraic simplification used (verified vs reference to 2.5e-6):
the rule-selection softmax input is
    score[b,h,t,r] = v_q . w_sel + bsc + attn[b,h,t,r,:] . w_vd
and the first two terms are constant in r, so they cancel in the softmax
over r. Wvq/bvq/Wsc[:, :SEL]/bsc never affect the output. Further, with
unnormalized attention A~_r = P~ @ v_r (P~ = exp(logits), d = P~ @ 1):
    g_r = (P~ @ u_r) / d        with u_r = v_r @ w_vd  (folded into V proj)
    sel = softmax_r(g);  out_h = sum_r (sel_r / d) * A~_r

Perf notes (v2 vs v1 baseline):
 - consolidated SBUF input tiles + one multi-dim DMA per tensor piece,
   spread over 4 engine queues (sync/scalar/gpsimd/vector) so the
   per-descriptor issue cost (~600ns) doesn't serialize the input stream
 - PE warm-up matmuls at t=0 so HAM reaches K=8/8 before real work and
   the projection matmuls never run at the cold 1.2 GHz clock
 - logits matmuls for head h+1 are emitted interleaved into the last
   t-chunk of head h's combine (and head 0's into the k-projection), so
   the per-head logits burst never stalls the PE on the exp activations
 - qp[h] (zero-padded per-head q) and oa[h] (transposed combine output)
   share one SBUF tile: qp dies exactly when oa's writes begin
"""

import numpy as np
from contextlib import ExitStack

import ml_dtypes
import concourse.bass as bass
import concourse.bacc as bacc
import concourse.mybir as mybir
from concourse import tile
from concourse.bass_utils import run_bass_kernel_spmd

AF = mybir.ActivationFunctionType
ALU = mybir.AluOpType
F32 = mybir.dt.float32

T, B, E, H, R = 2048, 2, 1024, 16, 8
HD, VD, SEL = 64, 128, 64
TS = T // 4            # 512 t-rows per core
NK = E // 128          # 8 contraction chunks over E
NS = T // 128          # 16 s-chunks
NT = TS // 128         # 4 t-chunks per core
VW = R * VD            # 1024 v columns
VX = VW + R            # 1032: v columns + 8 u columns
NCORES = 8
NWARM = 46             # PE warm-up matmuls (N=128) to get HAM to K=8/8

DT = mybir.dt.bfloat16
NPDT = ml_dtypes.bfloat16


def _build():
    nc = bacc.Bacc("TRN2", target_bir_lowering=False, debug=False,
                   num_devices=NCORES)
    qt = nc.declare_dram_parameter("qt_full", [E, T], DT, isOutput=False)
    qts_d = nc.declare_dram_parameter("qt_slice", [E, TS], DT, isOutput=False)
    wqt = nc.declare_dram_parameter("wqt", [E, E], DT, isOutput=False)
    bq_col = nc.declare_dram_parameter("bq_col", [E, 1], F32, isOutput=False)
    wkt = nc.declare_dram_parameter("wkt", [E, E], DT, isOutput=False)
    bk_col = nc.declare_dram_parameter("bk_col", [E, 1], F32, isOutput=False)
    wvt = nc.declare_dram_parameter("wvt", [E, VX], DT, isOutput=False)
    vbias = nc.declare_dram_parameter("vbias", [128, VX], F32, isOutput=False)
    wot = nc.declare_dram_parameter("wot", [H * VD, E], DT, isOutput=False)
    bo_bc = nc.declare_dram_parameter("bo_bc", [128, E], F32, isOutput=False)
    ident = nc.declare_dram_parameter("ident", [128, 128], DT, isOutput=False)
    out = nc.declare_dram_parameter("out", [TS, E], F32, isOutput=True)

    # DRAM views reshaped so one DMA covers all row-chunks: [p, chunk, col]
    qt_v = qt[:, :].rearrange("(k p) t -> p k t", k=NK)
    qts_v = qts_d[:, :].rearrange("(k p) t -> p k t", k=NK)
    wq_v = wqt[:, :].rearrange("(k p) e -> p k e", k=NK)
    wk_v = wkt[:, :].rearrange("(k p) e -> p k e", k=NK)
    wv_v = wvt[:, :].rearrange("(k p) e -> p k e", k=NK)
    wo_v = wot[:, :].rearrange("(k p) e -> p k e", k=H)
    bq_v = bq_col[:, :].rearrange("(k p) o -> p (k o)", k=NK)
    bk_v = bk_col[:, :].rearrange("(k p) o -> p (k o)", k=NK)

    with ExitStack() as ctx:
        tc = ctx.enter_context(tile.TileContext(nc))
        pers = ctx.enter_context(tc.tile_pool(name="pers", bufs=1))

        # ---- persistent SBUF tensors ----
        kt = [pers.tile([128, T], DT, tag=f"kt{m}", name=f"kt{m}") for m in range(NK)]
        # per-head zero-padded q (K=128 so FWL engages); after head h's
        # logits are done the same tile is re-used for oa[h] (the
        # transposed combine output feeding the out-projection lhsT)
        qoa = [pers.tile([128, TS], DT, tag=f"qoa{h}", name=f"qoa{h}") for h in range(H)]
        va = [pers.tile([128, VX + 1], DT, tag=f"va{s}", name=f"va{s}") for s in range(NS)]
        bq_sb = pers.tile([128, NK], F32, tag="bq", name="bq")
        bk_sb = pers.tile([128, NK], F32, tag="bk", name="bk")
        vb_sb = pers.tile([128, VX], F32, tag="vb", name="vb")
        bo_sb = pers.tile([128, E], F32, tag="bo", name="bo")
        id_sb = pers.tile([128, 128], DT, tag="id", name="id")
        wup = pers.tile([128, 128], DT, tag="wup", name="wup")
        dmagate = pers.tile([1, 8], DT, tag="dmagate", name="dmagate")

        # et (exp of logits) tiles cycle through this pool; 26 bufs >= 24
        # simultaneously-live tiles (16 of head h + 8 of head h+1 emitted
        # during head h's t=2 chunk, before any of head h's tiles free)
        exP = ctx.enter_context(tc.tile_pool(name="exP", bufs=26))
        # logits psum pool lives from the k-projection (head 0 interleave)
        # through all of phase B; SBUF and PSUM allocators are independent
        # so this outlives the phase-A SBUF staging pool below
        plt = ctx.enter_context(tc.tile_pool(name="plt", bufs=2,
                                             space="PSUM"))
        phA_ctx = tc.tile_pool(name="phA", bufs=1)
        phA = phA_ctx.__enter__()
        qts = phA.tile([128, NK, TS], DT, tag="qts", name="qts")
        qtall = phA.tile([128, NK, T], DT, tag="qtall", name="qtall")
        wqall = phA.tile([128, NK, E], DT, tag="wqall", name="wqall")
        wkall = phA.tile([128, NK, E], DT, tag="wkall", name="wkall")
        wvall = phA.tile([128, NK, VX], DT, tag="wvall", name="wvall")

        nc.vector.memset(wup[:], 0.0)

        # ---- input DMAs ----
        # The three hwdge queues share ~330 GB/s aggregate, and each queue
        # transfers strictly in issue order -- so order IS priority. The
        # critical chain is qts -> wq (q-proj) -> wk + qt quarters
        # (k-proj, nb-outer); wv/wo/biases are needed much later.
        # qts gates the very first matmul: split it across two queues
        nc.sync.dma_start(qts[:, 0:4, :], qts_v[:, 0:4, :])
        nc.gpsimd.dma_start(qts[:, 4:NK, :], qts_v[:, 4:NK, :])
        nc.scalar.dma_start(wqall[:, :, 0:128], wq_v[:, :, 0:128])
        nc.gpsimd.dma_start(bq_sb[:], bq_v)
        nc.gpsimd.dma_start(bk_sb[:], bk_v)
        nc.gpsimd.dma_start(id_sb[:], ident[:, :])
        nc.scalar.dma_start(wqall[:, :, 128:512], wq_v[:, :, 128:512])
        nc.scalar.dma_start(wqall[:, :, 512:768], wq_v[:, :, 512:768])
        nc.scalar.dma_start(wqall[:, :, 768:E], wq_v[:, :, 768:E])
        nc.sync.dma_start(wkall[:, :, 0:512], wk_v[:, :, 0:512])
        nc.gpsimd.dma_start(qtall[:, :, 0:TS], qt_v[:, :, 0:TS])
        nc.sync.dma_start(wkall[:, :, 512:E], wk_v[:, :, 512:E])
        nc.sync.dma_start(bo_sb[:], bo_bc[:, :])

        # ---- PE warm-up: get HAM to K=8/8 while input DMAs stream ----
        with tc.tile_pool(name="pwu", bufs=2, space="PSUM") as pwu:
            for i in range(NWARM):
                pw = pwu.tile([128, 128], F32, tag="pw", name="pw")
                nc.tensor.matmul(pw[:], lhsT=wup[:], rhs=wup[:],
                                 start=True, stop=True)

        # ---- logits emitter machinery ----
        plt_pool = [None]
        et_store = [[None] * NS for _ in range(H)]

        def logits_ops(h):
            m2 = h // 2
            ops = []
            for s in range(NS):
                def op(h=h, s=s, m2=m2):
                    psl = plt_pool[0].tile([128, TS], F32, tag="plt",
                                           name="psl")
                    nc.tensor.matmul(
                        psl[:],
                        lhsT=kt[m2][:, s * 128:(s + 1) * 128],
                        rhs=qoa[h][:],
                        start=True, stop=True)
                    et = exP.tile([128, TS], DT, tag="et", name="et")
                    nc.scalar.activation(et[:], psl[:], AF.Exp)
                    et_store[h][s] = et
                ops.append(op)
            return ops

        # ---- Phase A: projections ----
        plt_pool[0] = plt
        with tc.tile_pool(name="ppk", bufs=2, space="PSUM") as ppk:
            # q projection -> per-head zero-padded tiles
            for m in range(NK):
                c0, c1 = m * 128, (m + 1) * 128
                ps = ppk.tile([128, 512], F32, tag="ppk", name="ppk")
                for k in range(NK):
                    nc.tensor.matmul(
                        ps[:], lhsT=wqall[:, k, c0:c1],
                        rhs=qts[:, k, :],
                        start=(k == 0), stop=(k == NK - 1))
                h0, h1 = 2 * m, 2 * m + 1
                nc.vector.memset(qoa[h0][64:128, :], 0.0)
                nc.vector.memset(qoa[h1][0:64, :], 0.0)
                nc.scalar.activation(qoa[h0][0:64, :], ps[0:64, :],
                                     AF.Identity,
                                     bias=bq_sb[0:64, m:m + 1])
                nc.scalar.activation(qoa[h1][64:128, :], ps[64:128, :],
                                     AF.Identity,
                                     bias=bq_sb[64:128, m:m + 1])

            # the remaining bulk input (qt q2/q3, wv, vb: ~4.6MB, not
            # needed until ~60us+) is held back behind a marker op that
            # waits for a q-proj output: the 3 queues share ~330 GB/s,
            # so letting this bulk run at t=0 starves the critical
            # qts/wq stream (measured 14us PE stall)
            nc.gpsimd.tensor_copy(dmagate[:], qoa[5][0:1, 0:8])
            nc.gpsimd.dma_start(qtall[:, :, TS:2 * TS],
                                qt_v[:, :, TS:2 * TS])
            nc.gpsimd.dma_start(qtall[:, :, 2 * TS:3 * TS],
                                qt_v[:, :, 2 * TS:3 * TS])
            nc.gpsimd.dma_start(qtall[:, :, 3 * TS:T], qt_v[:, :, 3 * TS:T])
            nc.gpsimd.dma_start(wvall[:, :, 0:516], wv_v[:, :, 0:516])
            nc.gpsimd.dma_start(wvall[:, :, 516:VX], wv_v[:, :, 516:VX])
            nc.gpsimd.dma_start(vb_sb[:], vbias[:, :])

            # kT_all (E_out on partitions, s free); nb-outer so the pass
            # over all m-chunks starts after just ONE qt quarter arrives.
            # head-0 logits op s (s-chunk s, needing kt[0]'s nb=s//4
            # block) interleaves at (nb=s//4, m odd).
            h0ops = logits_ops(0)
            for nb in range(T // 512):
                for m in range(NK):
                    c0, c1 = m * 128, (m + 1) * 128
                    ps = ppk.tile([128, 512], F32, tag="ppk", name="ppk")
                    for k in range(NK):
                        nc.tensor.matmul(
                            ps[:], lhsT=wkall[:, k, c0:c1],
                            rhs=qtall[:, k, nb * 512:(nb + 1) * 512],
                            start=(k == 0), stop=(k == NK - 1))
                    nc.scalar.activation(
                        kt[m][:, nb * 512:(nb + 1) * 512],
                        ps[:], AF.Identity, bias=bk_sb[:, m:m + 1])
                    if m % 2 == 1 and h0ops:
                        h0ops.pop(0)()

        # v_all (s on partitions): [v | u] + bias, plus ones col
        with tc.tile_pool(name="ppv", bufs=2, space="PSUM") as ppv:
            for s in range(NS):
                c0, c1 = s * 128, (s + 1) * 128
                psv = ppv.tile([128, VX], F32, tag="ppv", name="ppv")
                for k in range(NK):
                    lhs = qtall[:, k, c0:c1]
                    nc.tensor.matmul(psv[:, 0:512], lhsT=lhs,
                                     rhs=wvall[:, k, 0:512],
                                     start=(k == 0), stop=(k == NK - 1))
                    nc.tensor.matmul(psv[:, 512:1024], lhsT=lhs,
                                     rhs=wvall[:, k, 512:1024],
                                     start=(k == 0), stop=(k == NK - 1))
                    nc.tensor.matmul(psv[:, 1024:VX], lhsT=lhs,
                                     rhs=wvall[:, k, 1024:VX],
                                     start=(k == 0), stop=(k == NK - 1))
                nc.vector.tensor_add(va[s][:, 0:VX], psv[:], vb_sb[:])
                nc.vector.memset(va[s][:, VX:VX + 1], 1.0)

        # phase-A SBUF staging (qt/wq/wk/wv, ~88KB/partition) dies here,
        # making room for wo + phase-B pools
        phA_ctx.__exit__(None, None, None)

        # ---- load Wo^T (needed in phase C; overlaps phase B) ----
        woP = ctx.enter_context(tc.tile_pool(name="woP", bufs=1))
        woall = woP.tile([128, H, E], DT, tag="wo", name="wo")
        nc.scalar.dma_start(woall[:, 0:8, :], wo_v[:, 0:8, :])
        nc.scalar.dma_start(woall[:, 8:H, :], wo_v[:, 8:H, :])

        # ---- Phase B: attention per head ----
        with (
            tc.tile_pool(name="pa", bufs=2, space="PSUM") as pa,
            tc.tile_pool(name="pas", bufs=1, space="PSUM") as pas,
            tc.tile_pool(name="pt", bufs=1, space="PSUM") as pt,
            tc.tile_pool(name="sm", bufs=4) as sm,
            tc.tile_pool(name="ocp", bufs=4) as ocp,
            tc.tile_pool(name="ob", bufs=2) as obp,
        ):
            def out_proj_group(t):
                # phase C folded into phase B: psums come from the (by
                # now idle) logits pool, and the t=0 group is emitted
                # before the final transpose flush so the PE never waits
                # on the last combine's DVE chain
                t0, t1 = t * 128, (t + 1) * 128
                for e in range(E // 512):
                    pso = plt.tile([128, 512], F32, tag="plt", name="pso")
                    for k in range(H):
                        nc.tensor.matmul(
                            pso[:], lhsT=qoa[k][:, t0:t1],
                            rhs=woall[:, k, e * 512:(e + 1) * 512],
                            start=(k == 0), stop=(k == H - 1))
                    ob = obp.tile([128, 512], F32, tag="ob", name="ob")
                    if t == NT - 1:
                        # last chunk: halve the bias-add/store so the
                        # final DMAs overlap the add and spread queues
                        engs = (nc.sync, nc.gpsimd) if e == 0 \
                            else (nc.scalar, nc.sync)
                        for half, heng in enumerate(engs):
                            c0h = e * 512 + half * 256
                            nc.vector.tensor_add(
                                ob[:, half * 256:(half + 1) * 256],
                                pso[:, half * 256:(half + 1) * 256],
                                bo_sb[:, c0h:c0h + 256])
                            heng.dma_start(
                                out[t0:t1, c0h:c0h + 256],
                                ob[:, half * 256:(half + 1) * 256])
                    else:
                        nc.vector.tensor_add(ob[:], pso[:],
                                             bo_sb[:, e * 512:(e + 1) * 512])
                        eng = nc.sync if e == 0 else nc.scalar
                        eng.dma_start(out[t0:t1, e * 512:(e + 1) * 512],
                                      ob[:])

            def issue_transpose(hh, tt, octile):
                ptr = pt.tile([128, 128], DT, tag="ptr", name="ptr")
                nc.tensor.transpose(ptr[:], octile[:], id_sb[:])
                # psum->sbuf copy on vector (gpsimd can't read PSUM):
                # keeps the scalar engine free for the exp activations
                # that gate the logits psum pool
                nc.vector.tensor_copy(
                    qoa[hh][:, tt * 128:(tt + 1) * 128], ptr[:])

            pending = []
            for h in range(H):
                nxt = logits_ops(h + 1) if h + 1 < H else []
                for t in range(NT):
                    t0, t1 = t * 128, (t + 1) * 128
                    psa = pa.tile([128, VW], F32, tag="psa", name="psa")
                    pss = pas.tile([128, R + 1], F32, tag="pss",
                                   name="pss")
                    for s in range(NS):
                        lhs = et_store[h][s][:, t0:t1]
                        st, sp = (s == 0), (s == NS - 1)
                        nc.tensor.matmul(psa[:, 0:512], lhsT=lhs,
                                         rhs=va[s][:, 0:512],
                                         start=st, stop=sp)
                        nc.tensor.matmul(psa[:, 512:1024], lhsT=lhs,
                                         rhs=va[s][:, 512:1024],
                                         start=st, stop=sp)
                        nc.tensor.matmul(pss[:], lhsT=lhs,
                                         rhs=va[s][:, 1024:VX + 1],
                                         start=st, stop=sp)
                        # next head's logits: 1 per 2 s-steps over the
                        # last two t-chunks (32 slots for 16 ops) so
                        # the ~670ns exp activations keep pace with
                        # the 2-bank logits psum pool
                        if t >= NT - 2 and s % 2 == 1 and nxt:
                            nxt.pop(0)()
                    # selection weights: w_r = softmax_r(G~_r/d) / d
                    rcp_d = sm.tile([128, 1], F32, tag="rcpd",
                                    name="rcpd")
                    nc.vector.reciprocal(rcp_d[:], pss[:, R:R + 1])
                    g = sm.tile([128, R], F32, tag="g", name="g")
                    nc.vector.tensor_scalar_mul(g[:], pss[:, 0:R],
                                                rcp_d[:])
                    selw = sm.tile([128, R], F32, tag="selw", name="selw")
                    ssum = sm.tile([128, 1], F32, tag="ssum", name="ssum")
                    nc.scalar.activation(selw[:], g[:], AF.Exp,
                                         accum_out=ssum[:])
                    den = sm.tile([128, 1], F32, tag="den", name="den")
                    nc.vector.tensor_scalar_mul(den[:], ssum[:],
                                                pss[:, R:R + 1])
                    rcp2 = sm.tile([128, 1], F32, tag="rcp2", name="rcp2")
                    nc.vector.reciprocal(rcp2[:], den[:])
                    w = sm.tile([128, R], F32, tag="w", name="w")
                    nc.vector.tensor_scalar_mul(w[:], selw[:], rcp2[:])
                    # combine rules: out_tile = sum_r w_r * A~_r
                    acc = sm.tile([128, 128], F32, tag="acc", name="acc")
                    nc.vector.tensor_scalar_mul(acc[:], psa[:, 0:128],
                                                w[:, 0:1])
                    for r in range(1, R - 1):
                        acc2 = sm.tile([128, 128], F32, tag="acc",
                                       name="acc")
                        nc.vector.scalar_tensor_tensor(
                            acc2[:], psa[:, r * 128:(r + 1) * 128],
                            w[:, r:r + 1], acc[:],
                            op0=ALU.mult, op1=ALU.add)
                        acc = acc2
                    octile = ocp.tile([128, VD], DT, tag="oc", name="oc")
                    nc.vector.scalar_tensor_tensor(
                        octile[:], psa[:, (R - 1) * 128:R * 128],
                        w[:, R - 1:R], acc[:], op0=ALU.mult, op1=ALU.add)
                    if pending:
                        issue_transpose(*pending.pop(0))
                    pending.append((h, t, octile))
            # t=0 out-projection first: it only needs the (already
            # copied) t=0 slices, and runs while the last combine's DVE
            # chain + final transpose drain
            out_proj_group(0)
            for hh, tt, octile in pending:
                issue_transpose(hh, tt, octile)
            for t in range(1, NT):
                out_proj_group(t)
    nc.finalize()
    return nc


_NC_CACHE = None


def _get_nc():
    global _NC_CACHE
    if _NC_CACHE is None:
        _NC_CACHE = _build()
    return _NC_CACHE


def _prep_in_maps(query, Wq, bq, Wk, bk, Wv, bv, Wsc, Wo, bo):
    scale = np.float32(HD ** -0.5)
    w_vd = Wsc[0, SEL:].astype(np.float32)          # (VD,)

    wqt = np.ascontiguousarray((Wq * scale).T).astype(NPDT)
    bq_col = (bq * scale).reshape(E, 1).astype(np.float32)
    wkt = np.ascontiguousarray(Wk.T).astype(NPDT)
    bk_col = bk.reshape(E, 1).astype(np.float32)

    WvT = np.ascontiguousarray(Wv.T).astype(np.float32)      # (E, VW)
    U_w = np.einsum("erd,d->er", WvT.reshape(E, R, VD), w_vd)  # (E, R)
    wvt = np.concatenate([WvT, U_w], axis=1).astype(NPDT)    # (E, VX)
    ubias = np.einsum("rd,d->r", bv.reshape(R, VD), w_vd)    # (R,)
    vb_row = np.concatenate([bv.astype(np.float32), ubias.astype(np.float32)])
    vbias = np.ascontiguousarray(
        np.broadcast_to(vb_row, (128, VX))).astype(np.float32)

    wot = np.ascontiguousarray(Wo.T).astype(NPDT)            # (H*VD, E)
    bo_bc = np.ascontiguousarray(
        np.broadcast_to(bo, (128, E))).astype(np.float32)
    ident = np.eye(128, dtype=NPDT)

    shared = dict(wqt=wqt, bq_col=bq_col, wkt=wkt, bk_col=bk_col, wvt=wvt,
                  vbias=vbias, wot=wot, bo_bc=bo_bc, ident=ident)

    in_maps = []
    for c in range(NCORES):
        b, tq = c // 4, c % 4
        qT = np.ascontiguousarray(query[:, b, :].T).astype(NPDT)  # (E, T)
        m = dict(shared)
        m["qt_full"] = qT
        m["qt_slice"] = np.ascontiguousarray(qT[:, tq * TS:(tq + 1) * TS])
        in_maps.append(m)
    return in_maps


def kernel(query, Wq, bq, Wk, bk, Wv, bv, Wvq, bvq, Wsc, bsc, Wo, bo,
           _trace=False, _tmpdir=None):
    query = np.asarray(query, np.float32)
    in_maps = _prep_in_maps(
        np.asarray(query, np.float32), np.asarray(Wq, np.float32),
        np.asarray(bq, np.float32), np.asarray(Wk, np.float32),
        np.asarray(bk, np.float32), np.asarray(Wv, np.float32),
        np.asarray(bv, np.float32), np.asarray(Wsc, np.float32),
        np.asarray(Wo, np.float32), np.asarray(bo, np.float32))
    nc = _get_nc()
    res = run_bass_kernel_spmd(nc, in_maps, list(range(NCORES)),
                               trace=_trace, tmpdir=_tmpdir)
    out = np.empty((T, B, E), np.float32)
    for c in range(NCORES):
        b, tq = c // 4, c % 4
        out[tq * TS:(tq + 1) * TS, b, :] = res.results[c]["out"]
    kernel._last_results = res
    return out


# revision 44
# speedup vs baseline: 1.0020x; 1.0008x over previous
"""CompositionalAttention Trainium2 kernel (8 NeuronCores, SPMD).

Shapes (hardcoded): query (T=2048, B=2, E=1024), H=16 heads, R=8 rules,
HD=64, VD=128. Output (T, B, E) float32.

Sharding: (batch x t-quarter) -> 8 cores. Core c handles b = c//4 and the
t-slice [tq*512, (tq+1)*512) with tq = c%4, computing ALL heads for that
slice so the output projection needs no cross-core reduction. Each core
returns its exact (512, 1024) slice of the final output.

Algebraic simplification used (verified vs reference to 2.5e-6):
the rule-selection softmax input is
    score[b,h,t,r] = v_q . w_sel + bsc + attn[b,h,t,r,:] . w_vd
and the first two terms are constant in r, so they cancel in the softmax
over r. Wvq/bvq/Wsc[:, :SEL]/bsc never affect the output. Further, with
unnormalized attention A~_r = P~ @ v_r (P~ = exp(logits), d = P~ @ 1):
    g_r = (P~ @ u_r) / d        with u_r = v_r @ w_vd  (folded into V proj)
    sel = softmax_r(g);  out_h = sum_r (sel_r / d) * A~_r

Perf notes (v2 vs v1 baseline):
 - consolidated SBUF input tiles + one multi-dim DMA per tensor piece,
   spread over 4 engine queues (sync/scalar/gpsimd/vector) so the
   per-descriptor issue cost (~600ns) doesn't serialize the input stream
 - PE warm-up matmuls at t=0 so HAM reaches K=8/8 before real work and
   the projection matmuls never run at the cold 1.2 GHz clock
 - logits matmuls for head h+1 are emitted interleaved into the last
   t-chunk of head h's combine (and head 0's into the k-projection), so
   the per-head logits burst never stalls the PE on the exp activations
 - qp[h] (zero-padded per-head q) and oa[h] (transposed combine output)
   share one SBUF tile: qp dies exactly when oa's writes begin
"""

import numpy as np
from contextlib import ExitStack

import ml_dtypes
import concourse.bass as bass
import concourse.bacc as bacc
import concourse.mybir as mybir
from concourse import tile
from concourse.bass_utils import run_bass_kernel_spmd

AF = mybir.ActivationFunctionType
ALU = mybir.AluOpType
F32 = mybir.dt.float32

T, B, E, H, R = 2048, 2, 1024, 16, 8
HD, VD, SEL = 64, 128, 64
TS = T // 4            # 512 t-rows per core
NK = E // 128          # 8 contraction chunks over E
NS = T // 128          # 16 s-chunks
NT = TS // 128         # 4 t-chunks per core
VW = R * VD            # 1024 v columns
VX = VW + R            # 1032: v columns + 8 u columns
NCORES = 8
NWARM = 56             # PE warm-up matmuls (N=128) to get HAM to K=8/8

DT = mybir.dt.bfloat16
NPDT = ml_dtypes.bfloat16


def _build():
    nc = bacc.Bacc("TRN2", target_bir_lowering=False, debug=False,
                   num_devices=NCORES)
    qt = nc.declare_dram_parameter("qt_full", [E, T], DT, isOutput=False)
    qts_d = nc.declare_dram_parameter("qt_slice", [E, TS], DT, isOutput=False)
    wqt = nc.declare_dram_parameter("wqt", [E, E], DT, isOutput=False)
    bq_col = nc.declare_dram_parameter("bq_col", [E, 1], F32, isOutput=False)
    wkt = nc.declare_dram_parameter("wkt", [E, E], DT, isOutput=False)
    bk_col = nc.declare_dram_parameter("bk_col", [E, 1], F32, isOutput=False)
    wvt = nc.declare_dram_parameter("wvt", [E, VX], DT, isOutput=False)
    vbias = nc.declare_dram_parameter("vbias", [128, VX], F32, isOutput=False)
    wot = nc.declare_dram_parameter("wot", [H * VD, E], DT, isOutput=False)
    bo_bc = nc.declare_dram_parameter("bo_bc", [128, E], F32, isOutput=False)
    ident = nc.declare_dram_parameter("ident", [128, 128], DT, isOutput=False)
    out = nc.declare_dram_parameter("out", [TS, E], F32, isOutput=True)

    # DRAM views reshaped so one DMA covers all row-chunks: [p, chunk, col]
    qt_v = qt[:, :].rearrange("(k p) t -> p k t", k=NK)
    qts_v = qts_d[:, :].rearrange("(k p) t -> p k t", k=NK)
    wq_v = wqt[:, :].rearrange("(k p) e -> p k e", k=NK)
    wk_v = wkt[:, :].rearrange("(k p) e -> p k e", k=NK)
    wv_v = wvt[:, :].rearrange("(k p) e -> p k e", k=NK)
    wo_v = wot[:, :].rearrange("(k p) e -> p k e", k=H)
    bq_v = bq_col[:, :].rearrange("(k p) o -> p (k o)", k=NK)
    bk_v = bk_col[:, :].rearrange("(k p) o -> p (k o)", k=NK)

    with ExitStack() as ctx:
        tc = ctx.enter_context(tile.TileContext(nc))
        pers = ctx.enter_context(tc.tile_pool(name="pers", bufs=1))

        # ---- persistent SBUF tensors ----
        kt = [pers.tile([128, T], DT, tag=f"kt{m}", name=f"kt{m}") for m in range(NK)]
        # per-head zero-padded q (K=128 so FWL engages); after head h's
        # logits are done the same tile is re-used for oa[h] (the
        # transposed combine output feeding the out-projection lhsT)
        qoa = [pers.tile([128, TS], DT, tag=f"qoa{h}", name=f"qoa{h}") for h in range(H)]
        va = [pers.tile([128, VX + 1], DT, tag=f"va{s}", name=f"va{s}") for s in range(NS)]
        bq_sb = pers.tile([128, NK], F32, tag="bq", name="bq")
        bk_sb = pers.tile([128, NK], F32, tag="bk", name="bk")
        vb_sb = pers.tile([128, VX], F32, tag="vb", name="vb")
        bo_sb = pers.tile([128, E], F32, tag="bo", name="bo")
        id_sb = pers.tile([128, 128], DT, tag="id", name="id")
        wup = pers.tile([128, 128], DT, tag="wup", name="wup")
        dmagate = pers.tile([1, 8], DT, tag="dmagate", name="dmagate")

        # et (exp of logits) tiles cycle through this pool; 26 bufs >= 24
        # simultaneously-live tiles (16 of head h + 8 of head h+1 emitted
        # during head h's t=2 chunk, before any of head h's tiles free)
        exP = ctx.enter_context(tc.tile_pool(name="exP", bufs=26))
        # logits psum pool lives from the k-projection (head 0 interleave)
        # through all of phase B; SBUF and PSUM allocators are independent
        # so this outlives the phase-A SBUF staging pool below
        plt = ctx.enter_context(tc.tile_pool(name="plt", bufs=2,
                                             space="PSUM"))
        phA_ctx = tc.tile_pool(name="phA", bufs=1)
        phA = phA_ctx.__enter__()
        qts = phA.tile([128, NK, TS], DT, tag="qts", name="qts")
        qtall = phA.tile([128, NK, T], DT, tag="qtall", name="qtall")
        wqall = phA.tile([128, NK, E], DT, tag="wqall", name="wqall")
        wkall = phA.tile([128, NK, E], DT, tag="wkall", name="wkall")
        wvall = phA.tile([128, NK, VX], DT, tag="wvall", name="wvall")

        nc.vector.memset(wup[:], 0.0)

        # ---- input DMAs ----
        # The three hwdge queues share ~330 GB/s aggregate, and each queue
        # transfers strictly in issue order -- so order IS priority. The
        # critical chain is qts -> wq (q-proj) -> wk + qt quarters
        # (k-proj, nb-outer); wv/wo/biases are needed much later.
        nc.sync.dma_start(qts[:], qts_v)
        nc.scalar.dma_start(wqall[:, :, 0:128], wq_v[:, :, 0:128])
        nc.gpsimd.dma_start(bq_sb[:], bq_v)
        nc.gpsimd.dma_start(bk_sb[:], bk_v)
        nc.gpsimd.dma_start(id_sb[:], ident[:, :])
        nc.scalar.dma_start(wqall[:, :, 128:512], wq_v[:, :, 128:512])
        nc.scalar.dma_start(wqall[:, :, 512:768], wq_v[:, :, 512:768])
        nc.scalar.dma_start(wqall[:, :, 768:E], wq_v[:, :, 768:E])
        nc.gpsimd.dma_start(qtall[:, :, 0:TS], qt_v[:, :, 0:TS])
        nc.sync.dma_start(wkall[:, :, 0:512], wk_v[:, :, 0:512])
        nc.gpsimd.dma_start(qtall[:, :, TS:2 * TS], qt_v[:, :, TS:2 * TS])
        nc.sync.dma_start(wkall[:, :, 512:E], wk_v[:, :, 512:E])
        nc.sync.dma_start(bo_sb[:], bo_bc[:, :])

        # ---- PE warm-up: get HAM to K=8/8 while input DMAs stream ----
        with tc.tile_pool(name="pwu", bufs=2, space="PSUM") as pwu:
            for i in range(NWARM):
                pw = pwu.tile([128, 128], F32, tag="pw", name="pw")
                nc.tensor.matmul(pw[:], lhsT=wup[:], rhs=wup[:],
                                 start=True, stop=True)

        # ---- logits emitter machinery ----
        plt_pool = [None]
        et_store = [[None] * NS for _ in range(H)]

        def logits_ops(h):
            m2 = h // 2
            ops = []
            for s in range(NS):
                def op(h=h, s=s, m2=m2):
                    psl = plt_pool[0].tile([128, TS], F32, tag="plt",
                                           name="psl")
                    nc.tensor.matmul(
                        psl[:],
                        lhsT=kt[m2][:, s * 128:(s + 1) * 128],
                        rhs=qoa[h][:],
                        start=True, stop=True)
                    et = exP.tile([128, TS], DT, tag="et", name="et")
                    nc.scalar.activation(et[:], psl[:], AF.Exp)
                    et_store[h][s] = et
                ops.append(op)
            return ops

        # ---- Phase A: projections ----
        plt_pool[0] = plt
        with tc.tile_pool(name="ppk", bufs=2, space="PSUM") as ppk:
            # q projection -> per-head zero-padded tiles
            for m in range(NK):
                c0, c1 = m * 128, (m + 1) * 128
                ps = ppk.tile([128, 512], F32, tag="ppk", name="ppk")
                for k in range(NK):
                    nc.tensor.matmul(
                        ps[:], lhsT=wqall[:, k, c0:c1],
                        rhs=qts[:, k, :],
                        start=(k == 0), stop=(k == NK - 1))
                h0, h1 = 2 * m, 2 * m + 1
                nc.vector.memset(qoa[h0][64:128, :], 0.0)
                nc.vector.memset(qoa[h1][0:64, :], 0.0)
                nc.scalar.activation(qoa[h0][0:64, :], ps[0:64, :],
                                     AF.Identity,
                                     bias=bq_sb[0:64, m:m + 1])
                nc.scalar.activation(qoa[h1][64:128, :], ps[64:128, :],
                                     AF.Identity,
                                     bias=bq_sb[64:128, m:m + 1])

            # the remaining bulk input (qt q2/q3, wv, vb: ~4.6MB, not
            # needed until ~60us+) is held back behind a marker op that
            # waits for a q-proj output: the 3 queues share ~330 GB/s,
            # so letting this bulk run at t=0 starves the critical
            # qts/wq stream (measured 14us PE stall)
            nc.gpsimd.tensor_copy(dmagate[:], qoa[5][0:1, 0:8])
            nc.gpsimd.dma_start(qtall[:, :, 2 * TS:3 * TS],
                                qt_v[:, :, 2 * TS:3 * TS])
            nc.gpsimd.dma_start(qtall[:, :, 3 * TS:T], qt_v[:, :, 3 * TS:T])
            nc.gpsimd.dma_start(wvall[:, :, 0:516], wv_v[:, :, 0:516])
            nc.gpsimd.dma_start(wvall[:, :, 516:VX], wv_v[:, :, 516:VX])
            nc.gpsimd.dma_start(vb_sb[:], vbias[:, :])

            # kT_all (E_out on partitions, s free); nb-outer so the pass
            # over all m-chunks starts after just ONE qt quarter arrives.
            # head-0 logits op s (s-chunk s, needing kt[0]'s nb=s//4
            # block) interleaves at (nb=s//4, m odd).
            h0ops = logits_ops(0)
            for nb in range(T // 512):
                for m in range(NK):
                    c0, c1 = m * 128, (m + 1) * 128
                    ps = ppk.tile([128, 512], F32, tag="ppk", name="ppk")
                    for k in range(NK):
                        nc.tensor.matmul(
                            ps[:], lhsT=wkall[:, k, c0:c1],
                            rhs=qtall[:, k, nb * 512:(nb + 1) * 512],
                            start=(k == 0), stop=(k == NK - 1))
                    nc.scalar.activation(
                        kt[m][:, nb * 512:(nb + 1) * 512],
                        ps[:], AF.Identity, bias=bk_sb[:, m:m + 1])
                    if m % 2 == 1 and h0ops:
                        h0ops.pop(0)()

        # v_all (s on partitions): [v | u] + bias, plus ones col
        with tc.tile_pool(name="ppv", bufs=2, space="PSUM") as ppv:
            for s in range(NS):
                c0, c1 = s * 128, (s + 1) * 128
                psv = ppv.tile([128, VX], F32, tag="ppv", name="ppv")
                for k in range(NK):
                    lhs = qtall[:, k, c0:c1]
                    nc.tensor.matmul(psv[:, 0:512], lhsT=lhs,
                                     rhs=wvall[:, k, 0:512],
                                     start=(k == 0), stop=(k == NK - 1))
                    nc.tensor.matmul(psv[:, 512:1024], lhsT=lhs,
                                     rhs=wvall[:, k, 512:1024],
                                     start=(k == 0), stop=(k == NK - 1))
                    nc.tensor.matmul(psv[:, 1024:VX], lhsT=lhs,
                                     rhs=wvall[:, k, 1024:VX],
                                     start=(k == 0), stop=(k == NK - 1))
                nc.vector.tensor_add(va[s][:, 0:VX], psv[:], vb_sb[:])
                nc.vector.memset(va[s][:, VX:VX + 1], 1.0)

        # phase-A SBUF staging (qt/wq/wk/wv, ~88KB/partition) dies here,
        # making room for wo + phase-B pools
        phA_ctx.__exit__(None, None, None)

        # ---- load Wo^T (needed in phase C; overlaps phase B) ----
        woP = ctx.enter_context(tc.tile_pool(name="woP", bufs=1))
        woall = woP.tile([128, H, E], DT, tag="wo", name="wo")
        nc.scalar.dma_start(woall[:, 0:8, :], wo_v[:, 0:8, :])
        nc.scalar.dma_start(woall[:, 8:H, :], wo_v[:, 8:H, :])

        # ---- Phase B: attention per head ----
        with (
            tc.tile_pool(name="pa", bufs=2, space="PSUM") as pa,
            tc.tile_pool(name="pas", bufs=1, space="PSUM") as pas,
            tc.tile_pool(name="pt", bufs=1, space="PSUM") as pt,
            tc.tile_pool(name="sm", bufs=4) as sm,
            tc.tile_pool(name="ocp", bufs=4) as ocp,
            tc.tile_pool(name="ob", bufs=2) as obp,
        ):
            def out_proj_group(t):
                # phase C folded into phase B: psums come from the (by
                # now idle) logits pool, and the t=0 group is emitted
                # before the final transpose flush so the PE never waits
                # on the last combine's DVE chain
                t0, t1 = t * 128, (t + 1) * 128
                for e in range(E // 512):
                    pso = plt.tile([128, 512], F32, tag="plt", name="pso")
                    for k in range(H):
                        nc.tensor.matmul(
                            pso[:], lhsT=qoa[k][:, t0:t1],
                            rhs=woall[:, k, e * 512:(e + 1) * 512],
                            start=(k == 0), stop=(k == H - 1))
                    ob = obp.tile([128, 512], F32, tag="ob", name="ob")
                    if t == NT - 1:
                        # last chunk: halve the bias-add/store so the
                        # final DMAs overlap the add and spread queues
                        engs = (nc.sync, nc.gpsimd) if e == 0 \
                            else (nc.scalar, nc.sync)
                        for half, heng in enumerate(engs):
                            c0h = e * 512 + half * 256
                            nc.vector.tensor_add(
                                ob[:, half * 256:(half + 1) * 256],
                                pso[:, half * 256:(half + 1) * 256],
                                bo_sb[:, c0h:c0h + 256])
                            heng.dma_start(
                                out[t0:t1, c0h:c0h + 256],
                                ob[:, half * 256:(half + 1) * 256])
                    else:
                        nc.vector.tensor_add(ob[:], pso[:],
                                             bo_sb[:, e * 512:(e + 1) * 512])
                        eng = nc.sync if e == 0 else nc.scalar
                        eng.dma_start(out[t0:t1, e * 512:(e + 1) * 512],
                                      ob[:])

            def issue_transpose(hh, tt, octile):
                ptr = pt.tile([128, 128], DT, tag="ptr", name="ptr")
                nc.tensor.transpose(ptr[:], octile[:], id_sb[:])
                # psum->sbuf copy on vector (gpsimd can't read PSUM):
                # keeps the scalar engine free for the exp activations
                # that gate the logits psum pool
                nc.vector.tensor_copy(
                    qoa[hh][:, tt * 128:(tt + 1) * 128], ptr[:])

            pending = []
            for h in range(H):
                nxt = logits_ops(h + 1) if h + 1 < H else []
                for t in range(NT):
                    t0, t1 = t * 128, (t + 1) * 128
                    psa = pa.tile([128, VW], F32, tag="psa", name="psa")
                    pss = pas.tile([128, R + 1], F32, tag="pss",
                                   name="pss")
                    for s in range(NS):
                        lhs = et_store[h][s][:, t0:t1]
                        st, sp = (s == 0), (s == NS - 1)
                        nc.tensor.matmul(psa[:, 0:512], lhsT=lhs,
                                         rhs=va[s][:, 0:512],
                                         start=st, stop=sp)
                        nc.tensor.matmul(psa[:, 512:1024], lhsT=lhs,
                                         rhs=va[s][:, 512:1024],
                                         start=st, stop=sp)
                        nc.tensor.matmul(pss[:], lhsT=lhs,
                                         rhs=va[s][:, 1024:VX + 1],
                                         start=st, stop=sp)
                        # next head's logits: 1 per 2 s-steps over the
                        # last two t-chunks (32 slots for 16 ops) so
                        # the ~670ns exp activations keep pace with
                        # the 2-bank logits psum pool
                        if t >= NT - 2 and s % 2 == 1 and nxt:
                            nxt.pop(0)()
                    # selection weights: w_r = softmax_r(G~_r/d) / d
                    rcp_d = sm.tile([128, 1], F32, tag="rcpd",
                                    name="rcpd")
                    nc.vector.reciprocal(rcp_d[:], pss[:, R:R + 1])
                    g = sm.tile([128, R], F32, tag="g", name="g")
                    nc.vector.tensor_scalar_mul(g[:], pss[:, 0:R],
                                                rcp_d[:])
                    selw = sm.tile([128, R], F32, tag="selw", name="selw")
                    ssum = sm.tile([128, 1], F32, tag="ssum", name="ssum")
                    nc.scalar.activation(selw[:], g[:], AF.Exp,
                                         accum_out=ssum[:])
                    den = sm.tile([128, 1], F32, tag="den", name="den")
                    nc.vector.tensor_scalar_mul(den[:], ssum[:],
                                                pss[:, R:R + 1])
                    rcp2 = sm.tile([128, 1], F32, tag="rcp2", name="rcp2")
                    nc.vector.reciprocal(rcp2[:], den[:])
                    w = sm.tile([128, R], F32, tag="w", name="w")
                    nc.vector.tensor_scalar_mul(w[:], selw[:], rcp2[:])
                    # combine rules: out_tile = sum_r w_r * A~_r
                    acc = sm.tile([128, 128], F32, tag="acc", name="acc")
                    nc.vector.tensor_scalar_mul(acc[:], psa[:, 0:128],
                                                w[:, 0:1])
                    for r in range(1, R - 1):
                        acc2 = sm.tile([128, 128], F32, tag="acc",
                                       name="acc")
                        nc.vector.scalar_tensor_tensor(
                            acc2[:], psa[:, r * 128:(r + 1) * 128],
                            w[:, r:r + 1], acc[:],
                            op0=ALU.mult, op1=ALU.add)
                        acc = acc2
                    octile = ocp.tile([128, VD], DT, tag="oc", name="oc")
                    nc.vector.scalar_tensor_tensor(
                        octile[:], psa[:, (R - 1) * 128:R * 128],
                        w[:, R - 1:R], acc[:], op0=ALU.mult, op1=ALU.add)
                    if pending:
                        issue_transpose(*pending.pop(0))
                    pending.append((h, t, octile))
            # t=0 out-projection first: it only needs the (already
            # copied) t=0 slices, and runs while the last combine's DVE
            # chain + final transpose drain
            out_proj_group(0)
            for hh, tt, octile in pending:
                issue_transpose(hh, tt, octile)
            for t in range(1, NT):
                out_proj_group(t)
    nc.finalize()
    return nc


_NC_CACHE = None


def _get_nc():
    global _NC_CACHE
    if _NC_CACHE is None:
        _NC_CACHE = _build()
    return _NC_CACHE


def _prep_in_maps(query, Wq, bq, Wk, bk, Wv, bv, Wsc, Wo, bo):
    scale = np.float32(HD ** -0.5)
    w_vd = Wsc[0, SEL:].astype(np.float32)          # (VD,)

    wqt = np.ascontiguousarray((Wq * scale).T).astype(NPDT)
    bq_col = (bq * scale).reshape(E, 1).astype(np.float32)
    wkt = np.ascontiguousarray(Wk.T).astype(NPDT)
    bk_col = bk.reshape(E, 1).astype(np.float32)

    WvT = np.ascontiguousarray(Wv.T).astype(np.float32)      # (E, VW)
    U_w = np.einsum("erd,d->er", WvT.reshape(E, R, VD), w_vd)  # (E, R)
    wvt = np.concatenate([WvT, U_w], axis=1).astype(NPDT)    # (E, VX)
    ubias = np.einsum("rd,d->r", bv.reshape(R, VD), w_vd)    # (R,)
    vb_row = np.concatenate([bv.astype(np.float32), ubias.astype(np.float32)])
    vbias = np.ascontiguousarray(
        np.broadcast_to(vb_row, (128, VX))).astype(np.float32)

    wot = np.ascontiguousarray(Wo.T).astype(NPDT)            # (H*VD, E)
    bo_bc = np.ascontiguousarray(
        np.broadcast_to(bo, (128, E))).astype(np.float32)
    ident = np.eye(128, dtype=NPDT)

    shared = dict(wqt=wqt, bq_col=bq_col, wkt=wkt, bk_col=bk_col, wvt=wvt,
                  vbias=vbias, wot=wot, bo_bc=bo_bc, ident=ident)

    in_maps = []
    for c in range(NCORES):
        b, tq = c // 4, c % 4
        qT = np.ascontiguousarray(query[:, b, :].T).astype(NPDT)  # (E, T)
        m = dict(shared)
        m["qt_full"] = qT
        m["qt_slice"] = np.ascontiguousarray(qT[:, tq * TS:(tq + 1) * TS])
        in_maps.append(m)
    return in_maps


def kernel(query, Wq, bq, Wk, bk, Wv, bv, Wvq, bvq, Wsc, bsc, Wo, bo,
           _trace=False, _tmpdir=None):
    query = np.asarray(query, np.float32)
    in_maps = _prep_in_maps(
        np.asarray(query, np.float32), np.asarray(Wq, np.float32),
        np.asarray(bq, np.float32), np.asarray(Wk, np.float32),
        np.asarray(bk, np.float32), np.asarray(Wv, np.float32),
        np.asarray(bv, np.float32), np.asarray(Wsc, np.float32),
        np.asarray(Wo, np.float32), np.asarray(bo, np.float32))
    nc = _get_nc()
    res = run_bass_kernel_spmd(nc, in_maps, list(range(NCORES)),
                               trace=_trace, tmpdir=_tmpdir)
    out = np.empty((T, B, E), np.float32)
    for c in range(NCORES):
        b, tq = c // 4, c % 4
        out[tq * TS:(tq + 1) * TS, b, :] = res.results[c]["out"]
    kernel._last_results = res
    return out


# revision 48
# speedup vs baseline: 1.0157x; 1.0136x over previous
"""CompositionalAttention Trainium2 kernel (8 NeuronCores, SPMD).

Shapes (hardcoded): query (T=2048, B=2, E=1024), H=16 heads, R=8 rules,
HD=64, VD=128. Output (T, B, E) float32.

Sharding: (batch x t-quarter) -> 8 cores. Core c handles b = c//4 and the
t-slice [tq*512, (tq+1)*512) with tq = c%4, computing ALL heads for that
slice so the output projection needs no cross-core reduction. Each core
returns its exact (512, 1024) slice of the final output.

Algebraic simplification used (verified vs reference to 2.5e-6):
the rule-selection softmax input is
    score[b,h,t,r] = v_q . w_sel + bsc + attn[b,h,t,r,:] . w_vd
and the first two terms are constant in r, so they cancel in the softmax
over r. Wvq/bvq/Wsc[:, :SEL]/bsc never affect the output. Further, with
unnormalized attention A~_r = P~ @ v_r (P~ = exp(logits), d = P~ @ 1):
    g_r = (P~ @ u_r) / d        with u_r = v_r @ w_vd  (folded into V proj)
    sel = softmax_r(g);  out_h = sum_r (sel_r / d) * A~_r

Perf notes (v2 vs v1 baseline):
 - consolidated SBUF input tiles + one multi-dim DMA per tensor piece,
   spread over 4 engine queues (sync/scalar/gpsimd/vector) so the
   per-descriptor issue cost (~600ns) doesn't serialize the input stream
 - PE warm-up matmuls at t=0 so HAM reaches K=8/8 before real work and
   the projection matmuls never run at the cold 1.2 GHz clock
 - logits matmuls for head h+1 are emitted interleaved into the last
   t-chunk of head h's combine (and head 0's into the k-projection), so
   the per-head logits burst never stalls the PE on the exp activations
 - qp[h] (zero-padded per-head q) and oa[h] (transposed combine output)
   share one SBUF tile: qp dies exactly when oa's writes begin
"""

import numpy as np
from contextlib import ExitStack

import ml_dtypes
import concourse.bass as bass
import concourse.bacc as bacc
import concourse.mybir as mybir
from concourse import tile
from concourse.bass_utils import run_bass_kernel_spmd

AF = mybir.ActivationFunctionType
ALU = mybir.AluOpType
F32 = mybir.dt.float32

T, B, E, H, R = 2048, 2, 1024, 16, 8
HD, VD, SEL = 64, 128, 64
TS = T // 4            # 512 t-rows per core
NK = E // 128          # 8 contraction chunks over E
NS = T // 128          # 16 s-chunks
NT = TS // 128         # 4 t-chunks per core
VW = R * VD            # 1024 v columns
VX = VW + R            # 1032: v columns + 8 u columns
NCORES = 8
NWARM = 56             # PE warm-up matmuls (N=128) to get HAM to K=8/8

DT = mybir.dt.bfloat16
NPDT = ml_dtypes.bfloat16


def _build():
    nc = bacc.Bacc("TRN2", target_bir_lowering=False, debug=False,
                   num_devices=NCORES)
    qt = nc.declare_dram_parameter("qt_full", [E, T], DT, isOutput=False)
    qts_d = nc.declare_dram_parameter("qt_slice", [E, TS], DT, isOutput=False)
    wqt = nc.declare_dram_parameter("wqt", [E, E], DT, isOutput=False)
    bq_col = nc.declare_dram_parameter("bq_col", [E, 1], F32, isOutput=False)
    wkt = nc.declare_dram_parameter("wkt", [E, E], DT, isOutput=False)
    bk_col = nc.declare_dram_parameter("bk_col", [E, 1], F32, isOutput=False)
    wvt = nc.declare_dram_parameter("wvt", [E, VX], DT, isOutput=False)
    vbias = nc.declare_dram_parameter("vbias", [128, VX], F32, isOutput=False)
    wot = nc.declare_dram_parameter("wot", [H * VD, E], DT, isOutput=False)
    bo_bc = nc.declare_dram_parameter("bo_bc", [128, E], F32, isOutput=False)
    ident = nc.declare_dram_parameter("ident", [128, 128], DT, isOutput=False)
    out = nc.declare_dram_parameter("out", [TS, E], F32, isOutput=True)

    # DRAM views reshaped so one DMA covers all row-chunks: [p, chunk, col]
    qt_v = qt[:, :].rearrange("(k p) t -> p k t", k=NK)
    qts_v = qts_d[:, :].rearrange("(k p) t -> p k t", k=NK)
    wq_v = wqt[:, :].rearrange("(k p) e -> p k e", k=NK)
    wk_v = wkt[:, :].rearrange("(k p) e -> p k e", k=NK)
    wv_v = wvt[:, :].rearrange("(k p) e -> p k e", k=NK)
    wo_v = wot[:, :].rearrange("(k p) e -> p k e", k=H)
    bq_v = bq_col[:, :].rearrange("(k p) o -> p (k o)", k=NK)
    bk_v = bk_col[:, :].rearrange("(k p) o -> p (k o)", k=NK)

    with ExitStack() as ctx:
        tc = ctx.enter_context(tile.TileContext(nc))
        pers = ctx.enter_context(tc.tile_pool(name="pers", bufs=1))

        # ---- persistent SBUF tensors ----
        kt = [pers.tile([128, T], DT, tag=f"kt{m}", name=f"kt{m}") for m in range(NK)]
        # per-head zero-padded q (K=128 so FWL engages); after head h's
        # logits are done the same tile is re-used for oa[h] (the
        # transposed combine output feeding the out-projection lhsT)
        qoa = [pers.tile([128, TS], DT, tag=f"qoa{h}", name=f"qoa{h}") for h in range(H)]
        va = [pers.tile([128, VX + 1], DT, tag=f"va{s}", name=f"va{s}") for s in range(NS)]
        bq_sb = pers.tile([128, NK], F32, tag="bq", name="bq")
        bk_sb = pers.tile([128, NK], F32, tag="bk", name="bk")
        vb_sb = pers.tile([128, VX], F32, tag="vb", name="vb")
        bo_sb = pers.tile([128, E], F32, tag="bo", name="bo")
        id_sb = pers.tile([128, 128], DT, tag="id", name="id")
        wup = pers.tile([128, 128], DT, tag="wup", name="wup")
        dmagate = pers.tile([1, 8], DT, tag="dmagate", name="dmagate")

        # et (exp of logits) tiles cycle through this pool; 26 bufs >= 24
        # simultaneously-live tiles (16 of head h + 8 of head h+1 emitted
        # during head h's t=2 chunk, before any of head h's tiles free)
        exP = ctx.enter_context(tc.tile_pool(name="exP", bufs=26))
        phA_ctx = tc.tile_pool(name="phA", bufs=1)
        phA = phA_ctx.__enter__()
        qts = phA.tile([128, NK, TS], DT, tag="qts", name="qts")
        qtall = phA.tile([128, NK, T], DT, tag="qtall", name="qtall")
        wqall = phA.tile([128, NK, E], DT, tag="wqall", name="wqall")
        wkall = phA.tile([128, NK, E], DT, tag="wkall", name="wkall")
        wvall = phA.tile([128, NK, VX], DT, tag="wvall", name="wvall")

        nc.vector.memset(wup[:], 0.0)

        # ---- input DMAs ----
        # The three hwdge queues share ~330 GB/s aggregate, and each queue
        # transfers strictly in issue order -- so order IS priority. The
        # critical chain is qts -> wq (q-proj) -> wk + qt quarters
        # (k-proj, nb-outer); wv/wo/biases are needed much later.
        # qts/wq stream per k-chunk with FULL rows: column-sliced DMAs
        # fragment into 256B runs (~505B packets, measured ~57GB/s on the
        # scalar queue); full-row chunks move 2KB runs. The k-outer
        # q-projection below consumes exactly one (qts, wq) chunk pair
        # per accumulation round, so dependencies stay fine-grained.
        nc.gpsimd.dma_start(bq_sb[:], bq_v)
        nc.gpsimd.dma_start(bk_sb[:], bk_v)
        nc.gpsimd.dma_start(id_sb[:], ident[:, :])
        for k in range(NK):
            nc.sync.dma_start(qts[:, k, :], qts_v[:, k, :])
            nc.scalar.dma_start(wqall[:, k, :], wq_v[:, k, :])
        nc.gpsimd.dma_start(qtall[:, :, 0:TS], qt_v[:, :, 0:TS])
        nc.sync.dma_start(wkall[:, :, 0:512], wk_v[:, :, 0:512])
        nc.gpsimd.dma_start(qtall[:, :, TS:2 * TS], qt_v[:, :, TS:2 * TS])
        nc.sync.dma_start(wkall[:, :, 512:E], wk_v[:, :, 512:E])
        nc.sync.dma_start(bo_sb[:], bo_bc[:, :])

        # ---- PE warm-up: get HAM to K=8/8 while input DMAs stream ----
        with tc.tile_pool(name="pwu", bufs=2, space="PSUM") as pwu:
            for i in range(NWARM):
                pw = pwu.tile([128, 128], F32, tag="pw", name="pw")
                nc.tensor.matmul(pw[:], lhsT=wup[:], rhs=wup[:],
                                 start=True, stop=True)

        # ---- logits emitter machinery ----
        plt_pool = [None]
        et_store = [[None] * NS for _ in range(H)]

        def logits_ops(h):
            m2 = h // 2
            ops = []
            for s in range(NS):
                def op(h=h, s=s, m2=m2):
                    psl = plt_pool[0].tile([128, TS], F32, tag="plt",
                                           name="psl")
                    nc.tensor.matmul(
                        psl[:],
                        lhsT=kt[m2][:, s * 128:(s + 1) * 128],
                        rhs=qoa[h][:],
                        start=True, stop=True)
                    et = exP.tile([128, TS], DT, tag="et", name="et")
                    nc.scalar.activation(et[:], psl[:], AF.Exp)
                    et_store[h][s] = et
                ops.append(op)
            return ops

        # ---- Phase A: projections ----
        # q projection, k-OUTER: all 8 m-chunk psums accumulate in
        # parallel across 8 banks, so round k depends only on the k-th
        # (qts, wq) chunk pair -- the PE starts as soon as the first
        # 0.4MB lands instead of waiting for the full 3MB
        with tc.tile_pool(name="ppq", bufs=1, space="PSUM") as ppq:
            psq = [ppq.tile([128, 512], F32, tag=f"ppq{m}", name=f"ppq{m}")
                   for m in range(NK)]
            for k in range(NK):
                for m in range(NK):
                    nc.tensor.matmul(
                        psq[m][:], lhsT=wqall[:, k, m * 128:(m + 1) * 128],
                        rhs=qts[:, k, :],
                        start=(k == 0), stop=(k == NK - 1))
            for m in range(NK):
                h0, h1 = 2 * m, 2 * m + 1
                nc.vector.memset(qoa[h0][64:128, :], 0.0)
                nc.vector.memset(qoa[h1][0:64, :], 0.0)
                nc.scalar.activation(qoa[h0][0:64, :], psq[m][0:64, :],
                                     AF.Identity,
                                     bias=bq_sb[0:64, m:m + 1])
                nc.scalar.activation(qoa[h1][64:128, :], psq[m][64:128, :],
                                     AF.Identity,
                                     bias=bq_sb[64:128, m:m + 1])

        # logits psum pool lives from the k-projection (head 0
        # interleave) through phase B and C
        plt = ctx.enter_context(tc.tile_pool(name="plt", bufs=2,
                                             space="PSUM"))
        plt_pool[0] = plt
        with tc.tile_pool(name="ppk", bufs=2, space="PSUM") as ppk:
            # the remaining bulk input (qt q2/q3, wv, vb: ~4.6MB, not
            # needed until ~60us+) is held back behind a marker op that
            # waits for a q-proj output: the 3 queues share ~330 GB/s,
            # so letting this bulk run at t=0 starves the critical
            # qts/wq stream (measured 14us PE stall)
            nc.gpsimd.tensor_copy(dmagate[:], qoa[5][0:1, 0:8])
            nc.gpsimd.dma_start(qtall[:, :, 2 * TS:3 * TS],
                                qt_v[:, :, 2 * TS:3 * TS])
            nc.gpsimd.dma_start(qtall[:, :, 3 * TS:T], qt_v[:, :, 3 * TS:T])
            nc.gpsimd.dma_start(wvall[:, :, 0:516], wv_v[:, :, 0:516])
            nc.gpsimd.dma_start(wvall[:, :, 516:VX], wv_v[:, :, 516:VX])
            nc.gpsimd.dma_start(vb_sb[:], vbias[:, :])

            # kT_all (E_out on partitions, s free); nb-outer so the pass
            # over all m-chunks starts after just ONE qt quarter arrives.
            # head-0 logits op s (s-chunk s, needing kt[0]'s nb=s//4
            # block) interleaves at (nb=s//4, m odd).
            h0ops = logits_ops(0)
            for nb in range(T // 512):
                for m in range(NK):
                    c0, c1 = m * 128, (m + 1) * 128
                    ps = ppk.tile([128, 512], F32, tag="ppk", name="ppk")
                    for k in range(NK):
                        nc.tensor.matmul(
                            ps[:], lhsT=wkall[:, k, c0:c1],
                            rhs=qtall[:, k, nb * 512:(nb + 1) * 512],
                            start=(k == 0), stop=(k == NK - 1))
                    nc.scalar.activation(
                        kt[m][:, nb * 512:(nb + 1) * 512],
                        ps[:], AF.Identity, bias=bk_sb[:, m:m + 1])
                    if m % 2 == 1 and h0ops:
                        h0ops.pop(0)()

        # v_all (s on partitions): [v | u] + bias, plus ones col
        with tc.tile_pool(name="ppv", bufs=2, space="PSUM") as ppv:
            for s in range(NS):
                c0, c1 = s * 128, (s + 1) * 128
                psv = ppv.tile([128, VX], F32, tag="ppv", name="ppv")
                for k in range(NK):
                    lhs = qtall[:, k, c0:c1]
                    nc.tensor.matmul(psv[:, 0:512], lhsT=lhs,
                                     rhs=wvall[:, k, 0:512],
                                     start=(k == 0), stop=(k == NK - 1))
                    nc.tensor.matmul(psv[:, 512:1024], lhsT=lhs,
                                     rhs=wvall[:, k, 512:1024],
                                     start=(k == 0), stop=(k == NK - 1))
                    nc.tensor.matmul(psv[:, 1024:VX], lhsT=lhs,
                                     rhs=wvall[:, k, 1024:VX],
                                     start=(k == 0), stop=(k == NK - 1))
                nc.vector.tensor_add(va[s][:, 0:VX], psv[:], vb_sb[:])
                nc.vector.memset(va[s][:, VX:VX + 1], 1.0)

        # phase-A SBUF staging (qt/wq/wk/wv, ~88KB/partition) dies here,
        # making room for wo + phase-B pools
        phA_ctx.__exit__(None, None, None)

        # ---- load Wo^T (needed in phase C; overlaps phase B) ----
        woP = ctx.enter_context(tc.tile_pool(name="woP", bufs=1))
        woall = woP.tile([128, H, E], DT, tag="wo", name="wo")
        nc.scalar.dma_start(woall[:, 0:8, :], wo_v[:, 0:8, :])
        nc.scalar.dma_start(woall[:, 8:H, :], wo_v[:, 8:H, :])

        # ---- Phase B: attention per head ----
        with (
            tc.tile_pool(name="pa", bufs=2, space="PSUM") as pa,
            tc.tile_pool(name="pas", bufs=1, space="PSUM") as pas,
            tc.tile_pool(name="pt", bufs=1, space="PSUM") as pt,
            tc.tile_pool(name="sm", bufs=4) as sm,
            tc.tile_pool(name="ocp", bufs=4) as ocp,
            tc.tile_pool(name="ob", bufs=2) as obp,
        ):
            def out_proj_group(t):
                # phase C folded into phase B: psums come from the (by
                # now idle) logits pool, and the t=0 group is emitted
                # before the final transpose flush so the PE never waits
                # on the last combine's DVE chain
                t0, t1 = t * 128, (t + 1) * 128
                for e in range(E // 512):
                    pso = plt.tile([128, 512], F32, tag="plt", name="pso")
                    for k in range(H):
                        nc.tensor.matmul(
                            pso[:], lhsT=qoa[k][:, t0:t1],
                            rhs=woall[:, k, e * 512:(e + 1) * 512],
                            start=(k == 0), stop=(k == H - 1))
                    ob = obp.tile([128, 512], F32, tag="ob", name="ob")
                    if t == NT - 1:
                        # last chunk: halve the bias-add/store so the
                        # final DMAs overlap the add and spread queues
                        engs = (nc.sync, nc.gpsimd) if e == 0 \
                            else (nc.scalar, nc.sync)
                        for half, heng in enumerate(engs):
                            c0h = e * 512 + half * 256
                            nc.vector.tensor_add(
                                ob[:, half * 256:(half + 1) * 256],
                                pso[:, half * 256:(half + 1) * 256],
                                bo_sb[:, c0h:c0h + 256])
                            heng.dma_start(
                                out[t0:t1, c0h:c0h + 256],
                                ob[:, half * 256:(half + 1) * 256])
                    else:
                        nc.vector.tensor_add(ob[:], pso[:],
                                             bo_sb[:, e * 512:(e + 1) * 512])
                        eng = nc.sync if e == 0 else nc.scalar
                        eng.dma_start(out[t0:t1, e * 512:(e + 1) * 512],
                                      ob[:])

            def issue_transpose(hh, tt, octile):
                ptr = pt.tile([128, 128], DT, tag="ptr", name="ptr")
                nc.tensor.transpose(ptr[:], octile[:], id_sb[:])
                # psum->sbuf copy on vector (gpsimd can't read PSUM):
                # keeps the scalar engine free for the exp activations
                # that gate the logits psum pool
                nc.vector.tensor_copy(
                    qoa[hh][:, tt * 128:(tt + 1) * 128], ptr[:])

            pending = []
            for h in range(H):
                nxt = logits_ops(h + 1) if h + 1 < H else []
                for t in range(NT):
                    t0, t1 = t * 128, (t + 1) * 128
                    psa = pa.tile([128, VW], F32, tag="psa", name="psa")
                    pss = pas.tile([128, R + 1], F32, tag="pss",
                                   name="pss")
                    for s in range(NS):
                        lhs = et_store[h][s][:, t0:t1]
                        st, sp = (s == 0), (s == NS - 1)
                        nc.tensor.matmul(psa[:, 0:512], lhsT=lhs,
                                         rhs=va[s][:, 0:512],
                                         start=st, stop=sp)
                        nc.tensor.matmul(psa[:, 512:1024], lhsT=lhs,
                                         rhs=va[s][:, 512:1024],
                                         start=st, stop=sp)
                        nc.tensor.matmul(pss[:], lhsT=lhs,
                                         rhs=va[s][:, 1024:VX + 1],
                                         start=st, stop=sp)
                        # next head's logits: 1 per 2 s-steps over the
                        # last two t-chunks (32 slots for 16 ops) so
                        # the ~670ns exp activations keep pace with
                        # the 2-bank logits psum pool
                        if t >= NT - 2 and s % 2 == 1 and nxt:
                            nxt.pop(0)()
                    # selection weights: w_r = softmax_r(G~_r/d) / d
                    rcp_d = sm.tile([128, 1], F32, tag="rcpd",
                                    name="rcpd")
                    nc.vector.reciprocal(rcp_d[:], pss[:, R:R + 1])
                    g = sm.tile([128, R], F32, tag="g", name="g")
                    nc.vector.tensor_scalar_mul(g[:], pss[:, 0:R],
                                                rcp_d[:])
                    selw = sm.tile([128, R], F32, tag="selw", name="selw")
                    ssum = sm.tile([128, 1], F32, tag="ssum", name="ssum")
                    nc.scalar.activation(selw[:], g[:], AF.Exp,
                                         accum_out=ssum[:])
                    den = sm.tile([128, 1], F32, tag="den", name="den")
                    nc.vector.tensor_scalar_mul(den[:], ssum[:],
                                                pss[:, R:R + 1])
                    rcp2 = sm.tile([128, 1], F32, tag="rcp2", name="rcp2")
                    nc.vector.reciprocal(rcp2[:], den[:])
                    w = sm.tile([128, R], F32, tag="w", name="w")
                    nc.vector.tensor_scalar_mul(w[:], selw[:], rcp2[:])
                    # combine rules: out_tile = sum_r w_r * A~_r
                    acc = sm.tile([128, 128], F32, tag="acc", name="acc")
                    nc.vector.tensor_scalar_mul(acc[:], psa[:, 0:128],
                                                w[:, 0:1])
                    for r in range(1, R - 1):
                        acc2 = sm.tile([128, 128], F32, tag="acc",
                                       name="acc")
                        nc.vector.scalar_tensor_tensor(
                            acc2[:], psa[:, r * 128:(r + 1) * 128],
                            w[:, r:r + 1], acc[:],
                            op0=ALU.mult, op1=ALU.add)
                        acc = acc2
                    octile = ocp.tile([128, VD], DT, tag="oc", name="oc")
                    nc.vector.scalar_tensor_tensor(
                        octile[:], psa[:, (R - 1) * 128:R * 128],
                        w[:, R - 1:R], acc[:], op0=ALU.mult, op1=ALU.add)
                    if pending:
                        issue_transpose(*pending.pop(0))
                    pending.append((h, t, octile))
            # t=0 out-projection first: it only needs the (already
            # copied) t=0 slices, and runs while the last combine's DVE
            # chain + final transpose drain
            out_proj_group(0)
            for hh, tt, octile in pending:
                issue_transpose(hh, tt, octile)
            for t in range(1, NT):
                out_proj_group(t)
    nc.finalize()
    return nc


_NC_CACHE = None


def _get_nc():
    global _NC_CACHE
    if _NC_CACHE is None:
        _NC_CACHE = _build()
    return _NC_CACHE


def _prep_in_maps(query, Wq, bq, Wk, bk, Wv, bv, Wsc, Wo, bo):
    scale = np.float32(HD ** -0.5)
    w_vd = Wsc[0, SEL:].astype(np.float32)          # (VD,)

    wqt = np.ascontiguousarray((Wq * scale).T).astype(NPDT)
    bq_col = (bq * scale).reshape(E, 1).astype(np.float32)
    wkt = np.ascontiguousarray(Wk.T).astype(NPDT)
    bk_col = bk.reshape(E, 1).astype(np.float32)

    WvT = np.ascontiguousarray(Wv.T).astype(np.float32)      # (E, VW)
    U_w = np.einsum("erd,d->er", WvT.reshape(E, R, VD), w_vd)  # (E, R)
    wvt = np.concatenate([WvT, U_w], axis=1).astype(NPDT)    # (E, VX)
    ubias = np.einsum("rd,d->r", bv.reshape(R, VD), w_vd)    # (R,)
    vb_row = np.concatenate([bv.astype(np.float32), ubias.astype(np.float32)])
    vbias = np.ascontiguousarray(
        np.broadcast_to(vb_row, (128, VX))).astype(np.float32)

    wot = np.ascontiguousarray(Wo.T).astype(NPDT)            # (H*VD, E)
    bo_bc = np.ascontiguousarray(
        np.broadcast_to(bo, (128, E))).astype(np.float32)
    ident = np.eye(128, dtype=NPDT)

    shared = dict(wqt=wqt, bq_col=bq_col, wkt=wkt, bk_col=bk_col, wvt=wvt,
                  vbias=vbias, wot=wot, bo_bc=bo_bc, ident=ident)

    in_maps = []
    for c in range(NCORES):
        b, tq = c // 4, c % 4
        qT = np.ascontiguousarray(query[:, b, :].T).astype(NPDT)  # (E, T)
        m = dict(shared)
        m["qt_full"] = qT
        m["qt_slice"] = np.ascontiguousarray(qT[:, tq * TS:(tq + 1) * TS])
        in_maps.append(m)
    return in_maps


def kernel(query, Wq, bq, Wk, bk, Wv, bv, Wvq, bvq, Wsc, bsc, Wo, bo,
           _trace=False, _tmpdir=None):
    query = np.asarray(query, np.float32)
    in_maps = _prep_in_maps(
        np.asarray(query, np.float32), np.asarray(Wq, np.float32),
        np.asarray(bq, np.float32), np.asarray(Wk, np.float32),
        np.asarray(bk, np.float32), np.asarray(Wv, np.float32),
        np.asarray(bv, np.float32), np.asarray(Wsc, np.float32),
        np.asarray(Wo, np.float32), np.asarray(bo, np.float32))
    nc = _get_nc()
    res = run_bass_kernel_spmd(nc, in_maps, list(range(NCORES)),
                               trace=_trace, tmpdir=_tmpdir)
    out = np.empty((T, B, E), np.float32)
    for c in range(NCORES):
        b, tq = c // 4, c % 4
        out[tq * TS:(tq + 1) * TS, b, :] = res.results[c]["out"]
    kernel._last_results = res
    return out
